# revision 1
# baseline (speedup 1.0000x reference)
"""LoRA linear kernel for Trainium2 (8 NeuronCores, SPMD data-parallel).

Computes y = x @ (B @ A)^T for
    x: [4, 2048, 4096] f32, B: [4096, 16] f32, A: [16, 4096] f32.

Strategy: never materialize W = B @ A.  Factor as t = x @ A^T (rank 16)
then y = t @ B^T.  Tokens (4*2048 = 8192) are sharded across 8 cores
(1024 tokens each); A and B are replicated.  The host pre-transposes x
into feature-major layout ([128, 32, TOK] = f-on-partitions) so every
device DMA is a perfectly contiguous pattern and the tensor engine can
contract over features directly.

Matmuls run in float32r (fp32 with 11-bit mantissa, single-pass PE mode,
4x the rate of 2-pass LOW_HIGH fp32).  Inputs are round-to-nearest
pre-rounded on the host so the truncation is unbiased.

Per-core dataflow:
  mm1: t^T[16, tok]  = sum_ko  A^T[ko]  (lhsT [128,16]) . x^T[ko] (rhs [128,tok])
  mm2: y[tok128, o]  = t^T[:, chunk] (lhsT [16,128])    . B^T     (rhs [16,512])
  y DMA'd out in natural token-major layout -> host just concatenates.
"""

import sys

import numpy as np

if "/opt/trn_rl_repo" not in sys.path:
    sys.path.insert(0, "/opt/trn_rl_repo")

# Problem shape (hardcoded per contract)
BATCH = 4
SEQ = 2048
D = 4096          # in_features == out_features
R = 16            # lora rank
NCORES = 8
NTOK = BATCH * SEQ            # 8192 tokens total
TOK = NTOK // NCORES          # 1024 tokens per core
P = 128                       # partitions
KO = D // P                   # 32 feature chunks
TB = 512                      # token block for mm1
NB = 512                      # matmul free dim for mm2 (fp32 max)
XC = 8                        # feature chunks per x DMA (2MB pieces)

# Module-level knobs for test.py (harness never touches these)
TRACE = False
LAST_RESULTS = None

_nc_cache = None


def _round_f32r(a):
    """Round fp32 array to f32r (11-bit mantissa) with round-to-nearest-even."""
    v = np.ascontiguousarray(a, dtype=np.float32).view(np.uint32)
    lsb = (v >> np.uint32(12)) & np.uint32(1)
    r = (v + np.uint32(0x7FF) + lsb) & np.uint32(0xFFFFF000)
    return r.view(np.float32)


def _build_program():
    from concourse import bacc, mybir, tile

    # Bacc (not raw Bass): its finalize() runs generate_event_semaphores,
    # which splits multi-sem waits to satisfy TRN2's 1-wait-per-instruction
    # hardware constraint (walrus rejects >1 otherwise).
    nc = bacc.Bacc(
        "TRN2", target_bir_lowering=False, debug=False, num_devices=NCORES
    )

    f32 = mybir.dt.float32
    f32r = mybir.dt.float32r

    xt = nc.dram_tensor("xt", [P, KO, TOK], f32r, kind="ExternalInput")
    at = nc.dram_tensor("at", [P, KO, R], f32r, kind="ExternalInput")
    bt = nc.dram_tensor("bt", [R, D], f32r, kind="ExternalInput")
    y = nc.dram_tensor("y", [TOK, D], f32, kind="ExternalOutput")

    with tile.TileContext(nc) as tc:
        with (
            tc.tile_pool(name="consts", bufs=1) as consts,
            tc.tile_pool(name="xin", bufs=12) as xin,
            tc.tile_pool(name="tbuf", bufs=2) as tbuf,
            tc.tile_pool(name="yout", bufs=2) as yout,
            tc.tile_pool(name="pt", bufs=2, space="PSUM") as pt_pool,
            tc.tile_pool(name="py", bufs=6, space="PSUM") as py_pool,
        ):
            at_s = consts.tile([P, KO, R], f32r)
            nc.sync.dma_start(at_s[:], at[:])
            bt_s = consts.tile([R, D], f32r)
            nc.sync.dma_start(bt_s[:], bt[:])

            # Warm-up matmuls: (a) make PE observe the at/bt DMA sems early,
            # (b) keep PE streaming during the x-DMA prologue so the HAM
            # clock gate reaches K=8/8 before the real matmuls start.
            obs1 = py_pool.tile([R, R], f32, tag="psum_y")
            nc.tensor.matmul(obs1[:], at_s[:, 0, :], at_s[:, 0, :R], start=True, stop=True)
            for _ in range(6):
                warm = py_pool.tile([P, NB], f32, tag="psum_y")
                nc.tensor.matmul(warm[:], bt_s[:, :P], bt_s[:, :NB], start=True, stop=True)
            tc.no_sync_barrier()

            # ko-chunks per x DMA: small first pieces so mm1 starts early
            XCS = [2, 2, 4, 4, 4, 4, 4, 4, 4]
            n_blocks = TOK // TB
            assert n_blocks == 2

            def load_x(tb):
                xts = []  # list of (tile, ko_base, width)
                ko_base = 0
                for w in XCS:
                    xt_tile = xin.tile([P, w, TB], f32r, tag="xt")
                    nc.sync.dma_start(
                        xt_tile[:],
                        xt[:, ko_base : ko_base + w, tb * TB : (tb + 1) * TB],
                    )
                    xts.append((xt_tile, ko_base, w))
                    ko_base += w
                return xts

            def mm1_range(xts, psum_t, ko_lo, ko_hi):
                for xt_tile, kb, w in xts:
                    for j in range(w):
                        ko = kb + j
                        if ko_lo <= ko < ko_hi:
                            nc.tensor.matmul(
                                psum_t[:],
                                at_s[:, ko, :],
                                xt_tile[:, j, :],
                                start=(ko == 0),
                                stop=(ko == KO - 1),
                            )

            def round_t(psum_t):
                # DVE copy fp32 -> f32r: the rounding step the verifier wants
                tT = tbuf.tile([R, TB], f32r)
                nc.vector.tensor_copy(tT[:], psum_t[:])
                return tT

            def mm2_chunk(tb, c, tT):
                y_row = yout.tile([P, D], f32)
                for n in range(D // NB):
                    psum_y = py_pool.tile([P, NB], f32, tag="psum_y")
                    nc.tensor.matmul(
                        psum_y[:],
                        tT[:, c * P : (c + 1) * P],
                        bt_s[:, n * NB : (n + 1) * NB],
                        start=True,
                        stop=True,
                    )
                    # Alternate PSUM-evacuation between DVE and ACT so
                    # neither engine gates the tensor engine's psum slots
                    if n % 3 == 2:
                        nc.scalar.copy(y_row[:, n * NB : (n + 1) * NB], psum_y[:])
                    else:
                        nc.vector.tensor_copy(y_row[:, n * NB : (n + 1) * NB], psum_y[:])
                row0 = tb * TB + c * P
                # scalar-engine HWDGE ring: offloads the Sync sequencer
                nc.scalar.dma_start(y[row0 : row0 + P, :], y_row[:])

            # PE order must follow x-arrival order (PE is FIFO: a matmul
            # waiting on a late DMA blocks everything behind it).
            for tb in range(n_blocks):
                xts = load_x(tb)
                psum_t = pt_pool.tile([R, TB], f32, tag="psum_t")
                mm1_range(xts, psum_t, 0, KO)
                tT = round_t(psum_t)
                for c in range(TB // P):
                    mm2_chunk(tb, c, tT)

    nc.finalize()
    return nc


def kernel(x, lora_matrix_B, lora_matrix_A):
    global _nc_cache, LAST_RESULTS
    from concourse.bass_utils import run_bass_kernel_spmd

    if _nc_cache is None:
        _nc_cache = _build_program()
    nc = _nc_cache

    x_flat = _round_f32r(np.asarray(x, dtype=np.float32)).reshape(NTOK, D)
    A = _round_f32r(np.asarray(lora_matrix_A, dtype=np.float32))
    B = _round_f32r(np.asarray(lora_matrix_B, dtype=np.float32))

    # at[p, ko, j] = A[j, ko*128 + p];  bt[j, o] = B[o, j]
    at_prep = np.ascontiguousarray(A.reshape(R, KO, P).transpose(2, 1, 0))
    bt_prep = np.ascontiguousarray(B.T)

    in_maps = []
    for c in range(NCORES):
        xc = x_flat[c * TOK : (c + 1) * TOK, :]
        # xt[p, ko, t] = xc[t, ko*128 + p]
        xt_prep = np.ascontiguousarray(xc.reshape(TOK, KO, P).transpose(2, 1, 0))
        in_maps.append({"xt": xt_prep, "at": at_prep, "bt": bt_prep})

    res = run_bass_kernel_spmd(
        nc, in_maps, core_ids=list(range(NCORES)), trace=TRACE
    )
    LAST_RESULTS = res

    y = np.concatenate([res.results[c]["y"] for c in range(NCORES)], axis=0)
    return y.reshape(BATCH, SEQ, D)



# revision 2
# speedup vs baseline: 1.2043x; 1.2043x over previous
"""LoRA linear kernel for Trainium2 (8 NeuronCores, SPMD data-parallel).

Computes y = x @ (B @ A)^T for
    x: [4, 2048, 4096] f32, B: [4096, 16] f32, A: [16, 4096] f32.

Strategy: never materialize W = B @ A.  Factor as t = x @ A^T (rank 16)
then y = t @ B^T.  Tokens (4*2048 = 8192) are sharded across 8 cores
(1024 tokens each); A and B are replicated.

All matmul operands are bf16 (host pre-rounded): the rank-16 contraction
keeps the bf16 rounding error ~1e-3, far inside the 2e-2 gate, and it
both halves the x HBM read (the kernel is DMA-bound) and runs the PE at
1 cycle/row (f32r measured ~3x slower).

Per-core dataflow, pipelined over 8 chunks of 128 tokens:
  mm1: t^T[16, 128]  = sum_ko  A^T[ko] (lhsT [128,16]) . x^T[ko] (rhs [128,128])
  mm2: y[tok128, o]  = t^T chunk (lhsT [16,128])       . B^T     (rhs [16,512])
  y DMA'd out token-major; host concatenates core shards.

x is staged host-side as [chunk][feat128][ko][tok128] so each chunk's
DMA is one fully-contiguous 1MB transfer (8KB per partition).  mm2 for
chunk c is emitted after mm1 for chunk c+1 so the PE never sits behind
the DVE round-trip that produces t^T.
"""

import sys

import numpy as np

if "/opt/trn_rl_repo" not in sys.path:
    sys.path.insert(0, "/opt/trn_rl_repo")

# Problem shape (hardcoded per contract)
BATCH = 4
SEQ = 2048
D = 4096          # in_features == out_features
R = 16            # lora rank
NCORES = 8
NTOK = BATCH * SEQ            # 8192 tokens total
TOK = NTOK // NCORES          # 1024 tokens per core
P = 128                       # partitions
KO = D // P                   # 32 feature chunks
CH = 128                      # tokens per pipeline chunk
NCH = TOK // CH               # 8 chunks per core
NB = 512                      # matmul free dim for mm2 (psum bank limit)

# Module-level knobs for test.py (harness never touches these)
TRACE = False
LAST_RESULTS = None

_nc_cache = None


def _build_program():
    from concourse import bacc, mybir, tile

    # Bacc (not raw Bass): its finalize() runs generate_event_semaphores,
    # which splits multi-sem waits to satisfy TRN2's 1-wait-per-instruction
    # hardware constraint (walrus rejects >1 otherwise).
    nc = bacc.Bacc(
        "TRN2", target_bir_lowering=False, debug=False, num_devices=NCORES
    )

    f32 = mybir.dt.float32
    bf16 = mybir.dt.bfloat16

    xt = nc.dram_tensor("xt", [NCH, P, KO, CH], bf16, kind="ExternalInput")
    at = nc.dram_tensor("at", [P, KO, R], bf16, kind="ExternalInput")
    bt = nc.dram_tensor("bt", [R, D], bf16, kind="ExternalInput")
    y = nc.dram_tensor("y", [TOK, D], f32, kind="ExternalOutput")

    with tile.TileContext(nc) as tc:
        with (
            tc.tile_pool(name="consts", bufs=1) as consts,
            tc.tile_pool(name="xin", bufs=NCH) as xin,
            tc.tile_pool(name="tbuf", bufs=3) as tbuf,
            tc.tile_pool(name="yout", bufs=5) as yout,
            tc.tile_pool(name="pt", bufs=2, space="PSUM") as pt_pool,
            tc.tile_pool(name="py", bufs=6, space="PSUM") as py_pool,
        ):
            at_s = consts.tile([P, KO, R], bf16)
            nc.sync.dma_start(at_s[:], at[:])
            bt_s = consts.tile([R, D], bf16)
            nc.sync.dma_start(bt_s[:], bt[:])

            # Warm-up matmuls: make PE observe the at/bt DMA sems early and
            # start the HAM clock-gate ramp during the x-DMA prologue.
            obs1 = py_pool.tile([R, R], f32, tag="psum_y")
            nc.tensor.matmul(obs1[:], at_s[:, 0, :], at_s[:, 0, :R], start=True, stop=True)
            for _ in range(2):
                warm = py_pool.tile([P, NB], f32, tag="psum_y")
                nc.tensor.matmul(warm[:], bt_s[:, :P], bt_s[:, :NB], start=True, stop=True)
            tc.no_sync_barrier()

            def load_x(c):
                xt_tile = xin.tile([P, KO, CH], bf16, tag="xt")
                nc.sync.dma_start(xt_tile[:], xt[c])
                return xt_tile

            def mm1(xt_tile):
                psum_t = pt_pool.tile([R, CH], f32, tag="psum_t")
                for ko in range(KO):
                    nc.tensor.matmul(
                        psum_t[:],
                        at_s[:, ko, :],
                        xt_tile[:, ko, :],
                        start=(ko == 0),
                        stop=(ko == KO - 1),
                    )
                # DVE copy psum f32 -> bf16 for the mm2 stationary operand
                tT = tbuf.tile([R, CH], bf16)
                nc.vector.tensor_copy(tT[:], psum_t[:])
                return tT

            def mm2(c, tT):
                y_row = yout.tile([P, D], f32)
                for n in range(D // NB):
                    psum_y = py_pool.tile([P, NB], f32, tag="psum_y")
                    nc.tensor.matmul(
                        psum_y[:],
                        tT[:],
                        bt_s[:, n * NB : (n + 1) * NB],
                        start=True,
                        stop=True,
                    )
                    # Alternate PSUM-evacuation between DVE and ACT so
                    # neither engine gates the tensor engine's psum slots
                    if n % 3 == 2:
                        nc.scalar.copy(y_row[:, n * NB : (n + 1) * NB], psum_y[:])
                    else:
                        nc.vector.tensor_copy(y_row[:, n * NB : (n + 1) * NB], psum_y[:])
                row0 = c * CH
                # scalar-engine HWDGE ring: offloads the Sync sequencer
                nc.scalar.dma_start(y[row0 : row0 + CH, :], y_row[:])

            # One-chunk software skew: mm2(c) is emitted after mm1(c+1), so
            # the PE FIFO never stalls on the DVE tT copy.  PE order still
            # follows x-arrival order (chunk order).
            tTs = {}
            tTs[0] = mm1(load_x(0))
            for c in range(1, NCH):
                tTs[c] = mm1(load_x(c))
                mm2(c - 1, tTs.pop(c - 1))
            mm2(NCH - 1, tTs.pop(NCH - 1))

    nc.finalize()
    return nc


def kernel(x, lora_matrix_B, lora_matrix_A):
    global _nc_cache, LAST_RESULTS
    import ml_dtypes
    from concourse.bass_utils import run_bass_kernel_spmd

    if _nc_cache is None:
        _nc_cache = _build_program()
    nc = _nc_cache

    bf16 = ml_dtypes.bfloat16
    x_flat = np.asarray(x, dtype=np.float32).reshape(NTOK, D).astype(bf16)
    A = np.asarray(lora_matrix_A, dtype=np.float32).astype(bf16)
    B = np.asarray(lora_matrix_B, dtype=np.float32).astype(bf16)

    # at[p, ko, j] = A[j, ko*128 + p];  bt[j, o] = B[o, j]
    at_prep = np.ascontiguousarray(A.reshape(R, KO, P).transpose(2, 1, 0))
    bt_prep = np.ascontiguousarray(B.T)

    in_maps = []
    for core in range(NCORES):
        xc = x_flat[core * TOK : (core + 1) * TOK, :]
        # xt[c, p, ko, t] = xc[c*128 + t, ko*128 + p]
        xt_prep = np.ascontiguousarray(
            xc.reshape(NCH, CH, KO, P).transpose(0, 3, 2, 1)
        )
        in_maps.append({"xt": xt_prep, "at": at_prep, "bt": bt_prep})

    res = run_bass_kernel_spmd(
        nc, in_maps, core_ids=list(range(NCORES)), trace=TRACE
    )
    LAST_RESULTS = res

    y = np.concatenate([res.results[c]["y"] for c in range(NCORES)], axis=0)
    return y.reshape(BATCH, SEQ, D)


# revision 3
# speedup vs baseline: 1.7513x; 1.4542x over previous
"""LoRA linear kernel for Trainium2 (8 NeuronCores, SPMD data-parallel).

Computes y = x @ (B @ A)^T for
    x: [4, 2048, 4096] f32, B: [4096, 16] f32, A: [16, 4096] f32.

Strategy: never materialize W = B @ A.  Factor as t = x @ A^T (rank 16)
then y = t @ B^T.  Tokens (4*2048 = 8192) are sharded across 8 cores
(1024 tokens each); A and B are replicated.

The kernel is HBM-DMA-bound, so both streams are bf16 on the wire:
  - x is cast to bf16 on the host (halves the read; rank-4096 contraction
    keeps the rounding error ~1e-3, far inside the 2e-2 gate);
  - y is written as bf16 and upcast to f32 on the host (halves the
    write; adds <=2^-9 relative error).
Per-core traffic is 8.4MB in + 8.4MB out vs 33.6MB for all-f32.

Per-core dataflow, 2 groups of 512 tokens, x DMA'd in 4x 1MB
fully-contiguous chunks per group (8 ko-slices each):
  mm1: t^T[16, 512]  = sum_ko  A^T[ko] (lhsT [128,16]) . x^T[ko] (rhs [128,512])
       accumulated incrementally as each 1MB chunk lands
  mm2: y[tok128, o]  = t^T[:, c*128:...] (lhsT [16,128]) . B^T (rhs [16,512])
  y DMA'd out token-major per 128-token row block (1MB bf16 transfers);
  host concatenates core shards and upcasts.
"""

import sys

import numpy as np

if "/opt/trn_rl_repo" not in sys.path:
    sys.path.insert(0, "/opt/trn_rl_repo")

# Problem shape (hardcoded per contract)
BATCH = 4
SEQ = 2048
D = 4096          # in_features == out_features
R = 16            # lora rank
NCORES = 8
NTOK = BATCH * SEQ            # 8192 tokens total
TOK = NTOK // NCORES          # 1024 tokens per core
P = 128                       # partitions
KO = D // P                   # 32 feature chunks
TB = 512                      # tokens per mm1 group (matmul free dim)
NG = TOK // TB                # 2 groups per core
NCHG = 4                      # x DMA chunks per group (1MB each)
KOC = KO // NCHG              # 8 ko-slices per chunk
NB = 512                      # matmul free dim for mm2 (psum bank limit)

# Module-level knobs for test.py (harness never touches these)
TRACE = False
LAST_RESULTS = None

_nc_cache = None


def _build_program():
    from concourse import bacc, mybir, tile

    # Bacc (not raw Bass): its finalize() runs generate_event_semaphores,
    # which splits multi-sem waits to satisfy TRN2's 1-wait-per-instruction
    # hardware constraint (walrus rejects >1 otherwise).
    nc = bacc.Bacc(
        "TRN2", target_bir_lowering=False, debug=False, num_devices=NCORES
    )

    f32 = mybir.dt.float32
    bf16 = mybir.dt.bfloat16

    xt = nc.dram_tensor("xt", [NG, NCHG, P, KOC, TB], bf16, kind="ExternalInput")
    at = nc.dram_tensor("at", [P, KO, R], bf16, kind="ExternalInput")
    bt = nc.dram_tensor("bt", [R, D], bf16, kind="ExternalInput")
    y = nc.dram_tensor("y", [TOK, D], bf16, kind="ExternalOutput")

    with tile.TileContext(nc) as tc:
        with (
            tc.tile_pool(name="consts", bufs=1) as consts,
            tc.tile_pool(name="xin", bufs=NG * NCHG) as xin,
            tc.tile_pool(name="tbuf", bufs=2) as tbuf,
            tc.tile_pool(name="yout", bufs=6) as yout,
            tc.tile_pool(name="pt", bufs=2, space="PSUM") as pt_pool,
            tc.tile_pool(name="py", bufs=6, space="PSUM") as py_pool,
        ):
            # consts ride the ACT HWDGE ring so the Sync ring starts
            # streaming x immediately
            at_s = consts.tile([P, KO, R], bf16)
            nc.scalar.dma_start(at_s[:], at[:])
            bt_s = consts.tile([R, D], bf16)
            nc.scalar.dma_start(bt_s[:], bt[:])

            # Warm-up matmuls: make PE observe the at/bt DMA sems early and
            # start the HAM clock-gate ramp during the x-DMA prologue.
            obs1 = py_pool.tile([R, R], f32, tag="psum_y")
            nc.tensor.matmul(obs1[:], at_s[:, 0, :], at_s[:, 0, :R], start=True, stop=True)
            for _ in range(2):
                warm = py_pool.tile([P, NB], f32, tag="psum_y")
                nc.tensor.matmul(warm[:], bt_s[:, :P], bt_s[:, :NB], start=True, stop=True)
            tc.no_sync_barrier()

            def process_group(g):
                # 4x 1MB fully-contiguous x chunks; mm1 accumulates over ko
                # incrementally as each chunk lands (PE order == arrival order)
                psum_t = pt_pool.tile([R, TB], f32, tag="psum_t")
                for c4 in range(NCHG):
                    xt_tile = xin.tile([P, KOC, TB], bf16, tag="xt")
                    nc.sync.dma_start(xt_tile[:], xt[g, c4])
                    for j in range(KOC):
                        ko = c4 * KOC + j
                        nc.tensor.matmul(
                            psum_t[:],
                            at_s[:, ko, :],
                            xt_tile[:, j, :],
                            start=(ko == 0),
                            stop=(ko == KO - 1),
                        )
                # DVE copy psum f32 -> bf16 for the mm2 stationary operand
                tT = tbuf.tile([R, TB], bf16)
                nc.vector.tensor_copy(tT[:], psum_t[:])

                for c in range(TB // P):
                    y_row = yout.tile([P, D], bf16)
                    for n in range(D // NB):
                        psum_y = py_pool.tile([P, NB], f32, tag="psum_y")
                        nc.tensor.matmul(
                            psum_y[:],
                            tT[:, c * P : (c + 1) * P],
                            bt_s[:, n * NB : (n + 1) * NB],
                            start=True,
                            stop=True,
                        )
                        # Alternate PSUM-evacuation between DVE and ACT so
                        # neither engine gates the tensor engine's psum slots
                        if n % 3 == 2:
                            nc.scalar.copy(y_row[:, n * NB : (n + 1) * NB], psum_y[:])
                        else:
                            nc.vector.tensor_copy(y_row[:, n * NB : (n + 1) * NB], psum_y[:])
                    row0 = g * TB + c * P
                    # scalar-engine HWDGE ring: offloads the Sync sequencer
                    nc.scalar.dma_start(y[row0 : row0 + P, :], y_row[:])

            for g in range(NG):
                process_group(g)

    nc.finalize()
    return nc


def kernel(x, lora_matrix_B, lora_matrix_A):
    global _nc_cache, LAST_RESULTS
    import ml_dtypes
    from concourse.bass_utils import run_bass_kernel_spmd

    if _nc_cache is None:
        _nc_cache = _build_program()
    nc = _nc_cache

    bf16 = ml_dtypes.bfloat16
    x_flat = np.asarray(x, dtype=np.float32).reshape(NTOK, D).astype(bf16)
    A = np.asarray(lora_matrix_A, dtype=np.float32).astype(bf16)
    B = np.asarray(lora_matrix_B, dtype=np.float32).astype(bf16)

    # at[p, ko, j] = A[j, ko*128 + p];  bt[j, o] = B[o, j]
    at_prep = np.ascontiguousarray(A.reshape(R, KO, P).transpose(2, 1, 0))
    bt_prep = np.ascontiguousarray(B.T)

    in_maps = []
    for core in range(NCORES):
        xc = x_flat[core * TOK : (core + 1) * TOK, :]
        # xt[g, c4, p, j, t] = xc[g*512 + t, (c4*8 + j)*128 + p]
        xt_prep = np.ascontiguousarray(
            xc.reshape(NG, TB, NCHG, KOC, P).transpose(0, 2, 4, 3, 1)
        )
        in_maps.append({"xt": xt_prep, "at": at_prep, "bt": bt_prep})

    res = run_bass_kernel_spmd(
        nc, in_maps, core_ids=list(range(NCORES)), trace=TRACE
    )
    LAST_RESULTS = res

    y = np.concatenate([res.results[c]["y"] for c in range(NCORES)], axis=0)
    return y.reshape(BATCH, SEQ, D).astype(np.float32)
